# revision 1
# baseline (speedup 1.0000x reference)
"""Bahdanau attention on 8 Trainium2 NeuronCores (Bass/Tile).

Problem:  S=2048, B=32, D=1024, E2=1024
  ws  = dec @ Wb.T                       [B, D]
  WH  = enc @ Wc.T                       [S, B, D]
  sc  = tanh(WH + ws) . Wa               [S, B]
  at  = softmax(sc, axis=0)              [S, B]
  out = einsum('sb,sbe->be', at, enc)[None]   [1, B, 2E]

Sharding: data-parallel over batch B across 8 cores (4 batches/core);
Wb/Wc/Wa replicated. Softmax axis (S) stays core-local.

enc is staged host-side in two layouts per core shard: natural [S, BL, E2]
(context contraction over s needs s on partitions) and transposed
[E2, BL, S] (the Wc matmul contracts over e, which must sit on partitions).
Both are DMA'd with per-partition-contiguous rows; the fp32->fp32r (TF32)
cast happens in the DMA so every matmul runs at 1 col/cycle.

Per-core dataflow:
  - WH^T [d-chunk=128p, s'] = (WcT chunk).T @ encT tile, accumulated over e
  - ACT: tanh(WH + wsT[d,b]) fused via per-partition bias
  - score row [1, s'] = sum_d WaChunk.T @ tanh_chunk  (PE)
  - exp on ACT (no max subtraction: |score| <= sum|Wa| ~ 26, exp fits fp32
    comfortably and softmax is shift-invariant), Z via activation accum_out
  - exp row -> column via a K=32 matmul against e0 (rows 1-31 zeroed)
  - context [1, e] += expCol.T @ enc_nat on PE across all s-tiles
    (unnormalized), scaled by 1/Z once per batch at the end.

Engines run their instruction streams in order, so emission order doubles as
a schedule: tile(0,0)'s enc DMAs are emitted before the weight DMAs to cover
DMA latency at kernel start.
"""

import numpy as np

S, B, D, E2 = 2048, 32, 1024, 1024
NCORES = 8
BL = B // NCORES          # batches per core
ST = 512                  # s-tile size
NST = S // ST             # s-tiles per batch
NSUB = ST // 128          # 128-row subtiles per s-tile
EC = E2 // 128            # e chunks
DC = D // 128             # d chunks

_CACHE = {}


def _build_nc():
    import concourse.bacc as bacc
    import concourse.tile as tile
    from concourse import mybir
    from concourse.masks import make_identity

    f32 = mybir.dt.float32
    f32r = mybir.dt.float32r
    f16 = mybir.dt.float16
    TANH = mybir.ActivationFunctionType.Tanh
    EXP = mybir.ActivationFunctionType.Exp
    X = mybir.AxisListType.X

    nc = bacc.Bacc()
    enc = nc.declare_dram_parameter("enc", [S, BL, E2], f32, isOutput=False)
    enct_h = nc.declare_dram_parameter("enct", [E2, BL, S], f32, isOutput=False)
    dect = nc.declare_dram_parameter("dect", [D, BL], f32, isOutput=False)
    wbt = nc.declare_dram_parameter("wbt", [D, D], f32, isOutput=False)    # Wb.T [d, d2]
    wct = nc.declare_dram_parameter("wct", [E2, D], f32, isOutput=False)   # Wc.T [e, d]
    wa2 = nc.declare_dram_parameter("wa2", [128, DC], f32, isOutput=False) # Wa chunks as cols
    outp = nc.declare_dram_parameter("out", [BL, E2], f32, isOutput=True)

    with tile.TileContext(nc) as tc:
        with (
            tc.tile_pool(name="const", bufs=1) as const_pool,
            tc.tile_pool(name="wbtp", bufs=1) as wbt_pool,
            tc.tile_pool(name="encn", bufs=3) as encn_pool,
            tc.tile_pool(name="enct", bufs=2) as enct_pool,
            tc.tile_pool(name="tanhp", bufs=4) as tanh_pool,
            tc.tile_pool(name="rows", bufs=2) as row_pool,
            tc.tile_pool(name="wh_ps", bufs=4, space="PSUM") as wh_ps,
            tc.tile_pool(name="sc_ps", bufs=2, space="PSUM") as sc_ps,
            tc.tile_pool(name="ctx_ps", bufs=2, space="PSUM") as ctx_ps,
        ):
            ex_ps = wh_ps  # columnize rides the wh rotation (same tag below)

            # ---- identity (fp32 via gpsimd) + fp16 e0 for the columnize ----
            id32 = const_pool.tile([128, 128], f32)
            make_identity(nc, id32)
            e0f16 = const_pool.tile([32, 1], f16)
            nc.vector.tensor_copy(out=e0f16, in_=id32[0:32, 0:1])

            # ---- emission helpers (order == per-engine schedule) ----
            # One fused dma_start per tile load: SWDGE descriptor generation
            # costs ~0.6us per dma_start on the GpSimd Q7, so batch the
            # subtiles into a single 2 MB transfer with a 3-D access pattern.
            def load_subs(bj, st, pfx=""):
                s0 = st * ST
                sub_big = encn_pool.tile([128, NSUB, E2], f32r, tag="sub",
                                         name=f"sub{pfx}")
                nc.gpsimd.dma_start(
                    out=sub_big,
                    in_=enc[s0:s0 + ST, bj, :].rearrange("(j p) e -> p j e", p=128),
                )
                return [sub_big[:, j, :] for j in range(NSUB)]

            def load_enct(bj, st, pfx=""):
                s0 = st * ST
                enct = []
                for ecn in range(EC):
                    et = enct_pool.tile([128, ST], f32r, tag="et", bufs=16,
                                        name=f"et{pfx}_{ecn}")
                    nc.gpsimd.dma_start(
                        out=et,
                        in_=enct_h[ecn * 128:(ecn + 1) * 128, bj, s0:s0 + ST],
                    )
                    enct.append(et)
                return enct

            # DMA queue order tuned for startup, strictly by first PE need:
            # ws inputs (dect tiny, then wbt chunk-paced), then the WH-path
            # tile(0,0) load, then Wc^T, then the ctx-path tile(0,0) load.
            dect_big = const_pool.tile([128, DC, BL], f32r)
            nc.gpsimd.dma_start(
                out=dect_big, in_=dect[:, :].rearrange("(c p) b -> p c b", p=128)
            )
            dect_sb = [dect_big[:, dk, :] for dk in range(DC)]
            wbt_sb = []
            for dk in range(DC):
                t = wbt_pool.tile([128, D], f32r, tag="wbt_sb", bufs=DC, name=f"wbt{dk}")
                nc.gpsimd.dma_start(out=t, in_=wbt[dk * 128:(dk + 1) * 128, :])
                wbt_sb.append(t)
            enct_cache = {(0, 0): load_enct(0, 0, pfx="00")}
            wct_sb = []
            for ecn in range(EC):
                t = const_pool.tile([128, D], f32r, tag="wct_sb", bufs=EC, name=f"wct{ecn}")
                nc.gpsimd.dma_start(out=t, in_=wct[ecn * 128:(ecn + 1) * 128, :])
                wct_sb.append(t)
            subs_cache = {(0, 0): load_subs(0, 0, pfx="00")}
            wa_sb = const_pool.tile([128, DC], f32r)
            nc.gpsimd.dma_start(out=wa_sb, in_=wa2[:, :])

            # ---- ws = dec @ Wb.T -> wsT [d2-chunk, b] for the tanh bias ----
            # ws natural [BL, d2]: lhsT = dectChunk [dk, BL], rhs = wbtChunk
            ws_sb = const_pool.tile([BL, D], f32r)
            ws_psum = [wh_ps.tile([BL, 512], f32, tag="wh", name=f"ws_psum{eh}")
                       for eh in range(2)]
            for dk in range(DC):
                for eh in range(2):
                    nc.tensor.matmul(
                        ws_psum[eh], dect_sb[dk], wbt_sb[dk][:, eh * 512:(eh + 1) * 512],
                        start=(dk == 0), stop=(dk == DC - 1),
                    )
            for eh in range(2):
                nc.scalar.copy(out=ws_sb[:, eh * 512:(eh + 1) * 512], in_=ws_psum[eh])
            # transpose ws -> wst chunks [128, BL] (tiny, K=BL transpose mode)
            wst_sb = []
            id32r = const_pool.tile([BL, BL], f32r)
            nc.vector.tensor_copy(out=id32r, in_=id32[0:BL, 0:BL])
            for dcn in range(DC):
                tp = wh_ps.tile([128, ST], f32r, tag="wh", name="tp_ws")
                nc.tensor.transpose(
                    tp[:, 0:BL], ws_sb[0:BL, dcn * 128:(dcn + 1) * 128], id32r
                )
                w = const_pool.tile([128, BL], f32, tag="wst_sb", bufs=DC, name=f"wst{dcn}")
                nc.vector.tensor_copy(out=w, in_=tp[:, 0:BL])
                wst_sb.append(w)

            # ---- main loop over (batch, s-tile) ----
            # The (exp-columnize + ctx) block of tile t is emitted after tile
            # t+1's WH/score work: the PE would otherwise idle ~1us per tile
            # waiting for ACT's exp. `pending` carries tile t's closure.
            state = {}   # per-b: exp_all, zparts, ctx
            pending = [] # [(bj, st, subs)]

            def emit_ctx(bj, st, subs):
                s0 = st * ST
                exp_all = state[bj]["exp_all"]
                ex = ex_ps.tile([128, NSUB], f32, tag="wh", name="ex")
                for j in range(NSUB):
                    nc.tensor.matmul(
                        ex[:, j:j + 1],
                        exp_all[0:32, s0 + j * 128:s0 + (j + 1) * 128],
                        e0f16,
                        start=True, stop=True,
                    )
                ext = row_pool.tile([128, NSUB], f32r, tag="ext", bufs=3)
                nc.vector.tensor_copy(out=ext, in_=ex)
                # per-tile psum group, folded into the SBUF accumulator by DVE
                ctx_acc = state[bj]["ctx_acc"]
                for eh in range(2):
                    ctx_t = ctx_ps.tile([1, 512], f32, tag="ctx", name="ctx_t")
                    for j in range(NSUB):
                        nc.tensor.matmul(
                            ctx_t,
                            ext[:, j:j + 1],
                            subs[j][:, eh * 512:(eh + 1) * 512],
                            start=(j == 0), stop=(j == NSUB - 1),
                        )
                    sl = ctx_acc[0:1, eh * 512:(eh + 1) * 512]
                    nc.vector.tensor_add(out=sl, in0=sl, in1=ctx_t)

            def finish_batch(bj):
                z = row_pool.tile([1, 1], f32, tag="z")
                nc.vector.reduce_sum(out=z, in_=state[bj]["zparts"], axis=X)
                rz = row_pool.tile([1, 1], f32, tag="rz")
                nc.vector.reciprocal(out=rz, in_=z)
                ctx_sb = row_pool.tile([1, E2], f32, tag="ctx_sb")
                nc.vector.tensor_scalar_mul(
                    out=ctx_sb, in0=state[bj]["ctx_acc"], scalar1=rz,
                )
                nc.sync.dma_start(out=outp[bj:bj + 1, :], in_=ctx_sb)

            for bj in range(BL):
                # [32, S] so the row->column move can be a K=32 matmul against
                # e0 (rows 1-31 are zero); only row 0 holds exp scores.
                # fp16 is safe here: scores are bounded well below fp16's
                # exp-overflow point (|score| <= ~5 for randn-scale inputs,
                # overflow needs >11), and fp16 rounding ~5e-4 matches the
                # TF32 precision used everywhere else.
                exp_all = row_pool.tile([32, S], f16, tag="exp_all")
                nc.vector.memset(exp_all, 0.0)
                zparts = row_pool.tile([1, NST], f32, tag="zparts")
                ctx_acc = row_pool.tile([1, E2], f32, tag="ctx_acc")
                nc.vector.memset(ctx_acc, 0.0)
                state[bj] = dict(exp_all=exp_all, zparts=zparts, ctx_acc=ctx_acc)

                for st in range(NST):
                    s0 = st * ST
                    subs = subs_cache.pop((bj, st), None) or load_subs(bj, st)
                    enct = enct_cache.pop((bj, st), None) or load_enct(bj, st)

                    # WH^T + tanh + score, d-chunks in pairs
                    sc = sc_ps.tile([1, ST], f32, tag="sc")
                    for dp in range(DC // 2):
                        whs = [wh_ps.tile([128, ST], f32, tag="wh", name=f"wh{dd}")
                               for dd in range(2)]
                        for ecn in range(EC):
                            for dd in range(2):
                                dcn = dp * 2 + dd
                                nc.tensor.matmul(
                                    whs[dd],
                                    wct_sb[ecn][:, dcn * 128:(dcn + 1) * 128],
                                    enct[ecn],
                                    start=(ecn == 0), stop=(ecn == EC - 1),
                                )
                        for dd in range(2):
                            dcn = dp * 2 + dd
                            th = tanh_pool.tile([128, ST], f32r, tag="th", name="th")
                            nc.scalar.activation(
                                out=th, in_=whs[dd], func=TANH,
                                bias=wst_sb[dcn][:, bj:bj + 1], scale=1.0,
                            )
                            nc.tensor.matmul(
                                sc, wa_sb[:, dcn:dcn + 1], th,
                                start=(dcn == 0), stop=(dcn == DC - 1),
                            )

                    # exp (+ per-tile partial of Z via accum_out)
                    nc.scalar.activation(
                        out=exp_all[0:1, s0:s0 + ST], in_=sc, func=EXP,
                        accum_out=zparts[0:1, st:st + 1],
                    )

                    # deferred ctx of the previous tile
                    if pending:
                        emit_ctx(*pending.pop())
                    pending.append((bj, st, subs))

                    if st == NST - 1 and bj > 0:
                        # previous batch is fully accumulated once its last
                        # pending ctx ran (one tile ago) -> normalize + store
                        finish_batch(bj - 1)

            emit_ctx(*pending.pop())
            finish_batch(BL - 1)

    nc.finalize()
    return nc


def _prep_inputs(dec_prev_hidden, enc_outputs, Wb, Wc, Wa):
    dec_prev_hidden = np.ascontiguousarray(np.asarray(dec_prev_hidden, dtype=np.float32))
    enc_outputs = np.ascontiguousarray(np.asarray(enc_outputs, dtype=np.float32))
    Wb = np.asarray(Wb, dtype=np.float32)
    Wc = np.asarray(Wc, dtype=np.float32)
    Wa = np.asarray(Wa, dtype=np.float32)

    wbt = np.ascontiguousarray(Wb.T)                     # [d, d2]
    wct = np.ascontiguousarray(Wc.T)                     # [e, d]
    dect = np.ascontiguousarray(dec_prev_hidden.T)       # [D, B]
    wa2 = np.ascontiguousarray(Wa.reshape(DC, 128).T)    # [128, DC]

    in_maps = []
    for i in range(NCORES):
        bsl = slice(i * BL, (i + 1) * BL)
        shard = enc_outputs[:, bsl, :]
        in_maps.append({
            "enc": np.ascontiguousarray(shard),
            "enct": np.ascontiguousarray(shard.transpose(2, 1, 0)),  # [E2, BL, S]
            "dect": np.ascontiguousarray(dect[:, bsl]),
            "wbt": wbt,
            "wct": wct,
            "wa2": wa2,
        })
    return in_maps


def _run(inputs, trace=False):
    from concourse.bass_utils import run_bass_kernel_spmd

    if "nc" not in _CACHE:
        _CACHE["nc"] = _build_nc()
    nc = _CACHE["nc"]
    in_maps = _prep_inputs(**inputs)
    res = run_bass_kernel_spmd(nc, in_maps, list(range(NCORES)), trace=trace)
    out = np.concatenate([res.results[i]["out"] for i in range(NCORES)], axis=0)
    return out[None, :, :].astype(np.float32), res


def kernel(dec_prev_hidden, enc_outputs, Wb, Wc, Wa):
    out, _ = _run(dict(
        dec_prev_hidden=dec_prev_hidden, enc_outputs=enc_outputs,
        Wb=Wb, Wc=Wc, Wa=Wa,
    ))
    return out

